# revision 3
# baseline (speedup 1.0000x reference)
"""Block-circulant process via truncated real-FFT factorization, v3.

out = x @ M through the 48-bin real FFT (B=128 blocks), all fp16,
fully SBUF-resident:
  stage A: sA[(c,e), (j,b)]  = F^T x_j       32 MMs, K=128   c=(fl,p)
  turn 1:  S2[(c,j), (e,b)]  = corner-turn   24 per-e SBUF DMAs
  stage M: mid[(c2,i),(e,b)] = Wm_e^T S2_e   24 MMs, K=128   c2=(q,fl)
  turn 2:  C2[(c2,e),(i,b)]  = corner-turn   32 per-i SBUF DMAs
  stage C: oT[t, (i,b)]      = G^T C2_i      32 MMs, K=96

Corner-turn calls write 96-128 partitions each (reads gather from 4
partitions on distinct AXI ports) so descriptors spread across all 16
SDMA engines and the HWDGE generator never backpressures. Host does
all layout permutes; x-in/out DMAs are partition-linear 8KB/partition.
Data-parallel over batch: 512 rows/core.
"""

import numpy as np

B = 128
KT = 48  # frequency truncation
KI = 32
KO = 32
BATCH = 4096
IN_F = 4096
OUT_F = 4096

N_CORES = 8
BQ = BATCH // N_CORES  # 512
NP = KT // 2  # 24 frequency pairs

_CACHE = {}
LAST_RESULTS = None
TRACE = False


def _build_nc():
    import concourse.bacc as bacc
    import concourse.mybir as mybir
    import concourse.tile as tile

    F16 = mybir.dt.float16
    F32 = mybir.dt.float32

    nc = bacc.Bacc(None, target_bir_lowering=False)
    xp = nc.declare_dram_parameter("xp", [128, KI * BQ], F16, isOutput=False)
    fg = nc.declare_dram_parameter("fg", [128, 256], F16, isOutput=False)
    wm = nc.declare_dram_parameter("wm", [128, NP * 128], F16, isOutput=False)
    op = nc.declare_dram_parameter("op", [128, KO * BQ], F16, isOutput=True)

    with tile.TileContext(nc) as tc:
        with (
            tc.tile_pool(name="cpool", bufs=1) as cpool,
            tc.tile_pool(name="psum", bufs=4, space="PSUM") as psum,
        ):
            fg_t = cpool.tile([128, 256], F16, name="fg_t")
            nc.gpsimd.dma_start(fg_t[:], fg[:])
            wm_t = cpool.tile([128, NP * 128], F16, name="wm_t")
            nc.gpsimd.dma_start(wm_t[:], wm[:])

            x_t = cpool.tile([128, KI * BQ], F16, name="x_t")
            chunks = [2, 2] + [4] * 7
            pos = 0
            for g, w in enumerate(chunks):
                (nc.sync if g % 2 == 0 else nc.scalar).dma_start(
                    x_t[:, pos * BQ:(pos + w) * BQ],
                    xp[:, pos * BQ:(pos + w) * BQ])
                pos += w

            sA = cpool.tile([96, KI * BQ], F16, name="sA")
            S2 = cpool.tile([128, NP * BQ], F16, name="S2")
            mid = cpool.tile([128, NP * BQ], F16, name="mid")
            C2 = cpool.tile([96, KO * BQ], F16, name="C2")
            oT = cpool.tile([128, KO * BQ], F16, name="oT")

            # corner-turn source views (4 partitions, distinct AXI ports)
            sAv = sA.rearrange("(c e) (j b) -> c e j b", c=4, j=KI)
            midv = mid.rearrange("(c i) (e b) -> c i e b", c=4, e=NP)

            f_ap = fg_t[:, 0:128]       # [t, (c,e)pad]  K=128
            g_ap = fg_t[0:96, 128:256]  # [(c2,e), t]    K=96

            # lane pattern: HWDGE pool (sync/scalar, shared 4-engine drain)
            # ~5/8 of calls, gpsimd (SWDGE, issue-bound) ~3/8
            hw_alt = [0]

            def turn_lane(k):
                if k % 8 < 5:
                    hw_alt[0] += 1
                    return nc.sync if hw_alt[0] % 2 == 0 else nc.scalar
                return nc.gpsimd

            # ---- PE warm-up during the x load (HAM clock ramp) ----
            for wn in range(4):
                ps = psum.tile([128, 2 * BQ], F32, name="ps_w", tag="ps")
                nc.tensor.matmul(ps[:, 0:256], f_ap, fg_t[:, 0:256],
                                 start=True, stop=True)

            # ---- stage A (copies on vector; turn-1 wave 1 for j<16 is
            # issued mid-stage: dst = contiguous S2 partitions 0:64) ----
            for j in range(0, KI, 2):
                ps = psum.tile([128, 2 * BQ], F32, name="ps_a", tag="ps")
                nc.tensor.matmul(ps[:, 0:BQ], f_ap,
                                 x_t[:, j * BQ:(j + 1) * BQ],
                                 start=True, stop=True)
                nc.tensor.matmul(ps[:, BQ:2 * BQ], f_ap,
                                 x_t[:, (j + 1) * BQ:(j + 2) * BQ],
                                 start=True, stop=True)
                nc.vector.tensor_copy(sA[:, j * BQ:(j + 2) * BQ], ps[0:96, :])
                if j == 14:
                    for e in range(NP):
                        turn_lane(e).dma_start(
                            S2[0:64, e * BQ:(e + 1) * BQ], sAv[:, e, 0:16])

            # ---- turn 1 wave 2 (j>=16 -> S2 partitions 64:128) ----
            for e in range(NP):
                turn_lane(e).dma_start(
                    S2[64:128, e * BQ:(e + 1) * BQ], sAv[:, e, 16:32])

            # ---- stage M (copies on vector; turn-2 wave 1 for e<12 is
            # issued mid-stage: dst = contiguous C2 partitions 0:48) ----
            for e in range(0, NP, 2):
                ps = psum.tile([128, 2 * BQ], F32, name="ps_m", tag="ps")
                nc.tensor.matmul(ps[:, 0:BQ], wm_t[:, e * 128:(e + 1) * 128],
                                 S2[:, e * BQ:(e + 1) * BQ],
                                 start=True, stop=True)
                nc.tensor.matmul(ps[:, BQ:2 * BQ],
                                 wm_t[:, (e + 1) * 128:(e + 2) * 128],
                                 S2[:, (e + 1) * BQ:(e + 2) * BQ],
                                 start=True, stop=True)
                if (e // 2) % 2 == 0:
                    nc.vector.tensor_copy(mid[:, e * BQ:(e + 2) * BQ], ps[:])
                else:
                    nc.scalar.copy(mid[:, e * BQ:(e + 2) * BQ], ps[:])
                if e == 10:
                    for i in range(KO):
                        turn_lane(i).dma_start(
                            C2[0:48, i * BQ:(i + 1) * BQ], midv[:, i, 0:12])

            # ---- turn 2 wave 2 (e>=12 -> C2 partitions 48:96) ----
            for i in range(KO):
                turn_lane(i).dma_start(
                    C2[48:96, i * BQ:(i + 1) * BQ], midv[:, i, 12:24])

            # ---- stage C + output ----
            for i in range(0, KO, 2):
                ps = psum.tile([128, 2 * BQ], F32, name="ps_c", tag="ps")
                nc.tensor.matmul(ps[:, 0:BQ], g_ap,
                                 C2[:, i * BQ:(i + 1) * BQ],
                                 start=True, stop=True)
                nc.tensor.matmul(ps[:, BQ:2 * BQ], g_ap,
                                 C2[:, (i + 1) * BQ:(i + 2) * BQ],
                                 start=True, stop=True)
                if (i // 2) % 2 == 0:
                    nc.vector.tensor_copy(oT[:, i * BQ:(i + 2) * BQ], ps[:])
                else:
                    nc.scalar.copy(oT[:, i * BQ:(i + 2) * BQ], ps[:])
                flush = {6: 0, 14: 8, 22: 16, 26: 24, 30: 28}
                if i in flush:
                    i0 = flush[i]
                    (nc.sync if i0 % 16 == 0 else nc.scalar).dma_start(
                        op[:, i0 * BQ:(i + 2) * BQ],
                        oT[:, i0 * BQ:(i + 2) * BQ])
    nc.finalize()
    return nc


def _get_nc():
    if "nc" not in _CACHE:
        _CACHE["nc"] = _build_nc()
    return _CACHE["nc"]


def _host_weights(W_real, W_imag):
    """F [128,128] (cols 96: zero), G [96,128], Wm [24,128,128] float64."""
    t = np.arange(B).astype(np.float64)
    # F columns (c, e) = (fl,p,e): col = fl*48 + p*24 + e; f = 2e+fl
    F = np.zeros((128, 128))
    for fl in range(2):
        for p in range(2):
            for e in range(NP):
                f = 2 * e + fl
                w = 2 * np.pi * f * t / B
                F[:, fl * 48 + p * 24 + e] = np.cos(w) if p == 0 else -np.sin(w)
    # G rows eh-blocked for contiguous turn-2 wave dsts:
    # row = eh*48 + (q*2+fl)*12 + el, e = eh*12+el, f = 2e+fl;
    # q=0 -> scale*cos, q=1 -> -scale*sin
    G = np.zeros((96, 128))
    scale = np.full(KT, 2.0 / B)
    scale[0] = 1.0 / B
    for eh in range(2):
        for q in range(2):
            for fl in range(2):
                for el in range(12):
                    f = 2 * (eh * 12 + el) + fl
                    w = 2 * np.pi * f * t / B
                    G[eh * 48 + (q * 2 + fl) * 12 + el] = (
                        scale[f] * np.cos(w) if q == 0
                        else -scale[f] * np.sin(w))
    # Wm[e]: rows (fl, p, j) = fl*64+p*32+j; cols (q, fl, i) = q*64+fl*32+i
    Wr = W_real.astype(np.float64)
    Wi = W_imag.astype(np.float64)
    Wm = np.zeros((NP, 128, 128))
    for e in range(NP):
        for fl in range(2):
            f = 2 * e + fl
            r0 = fl * 64
            c0 = fl * 32
            Wrf = Wr[:, :, f].T  # [j, i]
            Wif = Wi[:, :, f].T
            Wm[e, r0:r0 + 32, c0:c0 + 32] = Wrf            # p0 -> q0: Wr
            Wm[e, r0 + 32:r0 + 64, c0:c0 + 32] = Wif       # p1 -> q0: Wi
            Wm[e, r0:r0 + 32, 64 + c0:64 + c0 + 32] = -Wif  # p0 -> q1: -Wi
            Wm[e, r0 + 32:r0 + 64, 64 + c0:64 + c0 + 32] = Wrf  # p1 -> q1
    return F, G, Wm


def kernel(x, W_real, W_imag):
    global LAST_RESULTS
    from concourse.bass_utils import run_bass_kernel_spmd

    x = np.asarray(x, dtype=np.float32)
    F, G, Wm = _host_weights(np.asarray(W_real), np.asarray(W_imag))
    fg_pack = np.zeros((128, 256), np.float16)
    fg_pack[:, 0:128] = F.astype(np.float16)
    fg_pack[0:96, 128:256] = G.astype(np.float16)
    # S2 rows are jh-blocked (row = jh*64 + c*16 + jl, j = jh*16+jl) so
    # turn-1 waves write contiguous partitions; permute Wm rows to match
    rperm = np.empty(128, np.int64)
    for fl in range(2):
        for p in range(2):
            for j in range(KI):
                rperm[(j // 16) * 64 + (fl * 2 + p) * 16 + (j % 16)] = (
                    fl * 64 + p * 32 + j)
    wm_pack = np.ascontiguousarray(
        Wm[:, rperm, :].transpose(1, 0, 2)).reshape(
            128, NP * 128).astype(np.float16)
    x16 = x.astype(np.float16)

    in_maps = []
    for c in range(N_CORES):
        xs = x16[c * BQ:(c + 1) * BQ, :]  # [512, 4096]
        xpk = np.ascontiguousarray(
            xs.reshape(BQ, KI, B).transpose(2, 1, 0)).reshape(128, KI * BQ)
        in_maps.append({"xp": xpk, "fg": fg_pack, "wm": wm_pack})

    nc = _get_nc()
    res = run_bass_kernel_spmd(nc, in_maps, list(range(N_CORES)), trace=TRACE)
    LAST_RESULTS = res

    out = np.empty((BATCH, OUT_F), np.float32)
    for c in range(N_CORES):
        o = np.asarray(res.results[c]["op"])  # [128, KO*BQ] fp16
        out[c * BQ:(c + 1) * BQ, :] = (
            o.reshape(128, KO, BQ).transpose(2, 1, 0)
            .reshape(BQ, OUT_F).astype(np.float32))
    return out


# revision 4
# speedup vs baseline: 1.0105x; 1.0105x over previous
"""Block-circulant process via truncated real-FFT factorization, v3.

out = x @ M through the 48-bin real FFT (B=128 blocks), all fp16,
fully SBUF-resident:
  stage A: sA[(c,e), (j,b)]  = F^T x_j       32 MMs, K=128   c=(fl,p)
  turn 1:  S2[(c,j), (e,b)]  = corner-turn   24 per-e SBUF DMAs
  stage M: mid[(c2,i),(e,b)] = Wm_e^T S2_e   24 MMs, K=128   c2=(q,fl)
  turn 2:  C2[(c2,e),(i,b)]  = corner-turn   32 per-i SBUF DMAs
  stage C: oT[t, (i,b)]      = G^T C2_i      32 MMs, K=96

Corner-turn calls write 96-128 partitions each (reads gather from 4
partitions on distinct AXI ports) so descriptors spread across all 16
SDMA engines and the HWDGE generator never backpressures. Host does
all layout permutes; x-in/out DMAs are partition-linear 8KB/partition.
Data-parallel over batch: 512 rows/core.
"""

import numpy as np

B = 128
KT = 48  # frequency truncation
KI = 32
KO = 32
BATCH = 4096
IN_F = 4096
OUT_F = 4096

N_CORES = 8
BQ = BATCH // N_CORES  # 512
NP = KT // 2  # 24 frequency pairs

_CACHE = {}
LAST_RESULTS = None
TRACE = False


def _build_nc():
    import concourse.bacc as bacc
    import concourse.mybir as mybir
    import concourse.tile as tile

    F16 = mybir.dt.float16
    F32 = mybir.dt.float32

    nc = bacc.Bacc(None, target_bir_lowering=False)
    xp = nc.declare_dram_parameter("xp", [128, KI * BQ], F16, isOutput=False)
    fg = nc.declare_dram_parameter("fg", [128, 256], F16, isOutput=False)
    wm = nc.declare_dram_parameter("wm", [128, NP * 128], F16, isOutput=False)
    op = nc.declare_dram_parameter("op", [128, KO * BQ], F16, isOutput=True)

    with tile.TileContext(nc) as tc:
        with (
            tc.tile_pool(name="cpool", bufs=1) as cpool,
            tc.tile_pool(name="psum", bufs=4, space="PSUM") as psum,
        ):
            fg_t = cpool.tile([128, 256], F16, name="fg_t")
            nc.sync.dma_start(fg_t[:], fg[:])
            wm_t = cpool.tile([128, NP * 128], F16, name="wm_t")
            nc.gpsimd.dma_start(wm_t[:], wm[:])

            x_t = cpool.tile([128, KI * BQ], F16, name="x_t")
            chunks = [2, 2] + [4] * 7
            pos = 0
            for g, w in enumerate(chunks):
                (nc.sync if g % 2 == 0 else nc.scalar).dma_start(
                    x_t[:, pos * BQ:(pos + w) * BQ],
                    xp[:, pos * BQ:(pos + w) * BQ])
                pos += w

            sA = cpool.tile([96, KI * BQ], F16, name="sA")
            S2 = cpool.tile([128, NP * BQ], F16, name="S2")
            mid = cpool.tile([128, NP * BQ], F16, name="mid")
            C2 = cpool.tile([96, KO * BQ], F16, name="C2")
            oT = cpool.tile([128, KO * BQ], F16, name="oT")

            # corner-turn source views (4 partitions, distinct AXI ports)
            sAv = sA.rearrange("(c e) (j b) -> c e j b", c=4, j=KI)
            midv = mid.rearrange("(c i) (e b) -> c i e b", c=4, e=NP)

            f_ap = fg_t[:, 0:128]       # [t, (c,e)pad]  K=128
            g_ap = fg_t[0:96, 128:256]  # [(c2,e), t]    K=96

            # lane pattern: HWDGE pool (sync/scalar, shared 4-engine drain)
            # ~5/8 of calls, gpsimd (SWDGE, issue-bound) ~3/8
            hw_alt = [0]

            def turn_lane(k):
                if k % 8 < 5:
                    hw_alt[0] += 1
                    return nc.sync if hw_alt[0] % 2 == 0 else nc.scalar
                return nc.gpsimd

            # ---- PE warm-up during the x load (HAM clock ramp) ----
            for wn in range(2):
                ps = psum.tile([128, 2 * BQ], F32, name="ps_w", tag="ps")
                nc.tensor.matmul(ps[:, 0:256], f_ap, fg_t[:, 0:256],
                                 start=True, stop=True)

            # ---- stage A (copies on vector; turn-1 wave 1 for j<16 is
            # issued mid-stage: dst = contiguous S2 partitions 0:64) ----
            for j in range(0, KI, 2):
                ps = psum.tile([128, 2 * BQ], F32, name="ps_a", tag="ps")
                nc.tensor.matmul(ps[:, 0:BQ], f_ap,
                                 x_t[:, j * BQ:(j + 1) * BQ],
                                 start=True, stop=True)
                nc.tensor.matmul(ps[:, BQ:2 * BQ], f_ap,
                                 x_t[:, (j + 1) * BQ:(j + 2) * BQ],
                                 start=True, stop=True)
                nc.vector.tensor_copy(sA[:, j * BQ:(j + 2) * BQ], ps[0:96, :])
                if j == 14:
                    for e in range(NP):
                        turn_lane(e).dma_start(
                            S2[0:64, e * BQ:(e + 1) * BQ], sAv[:, e, 0:16])

            # ---- turn 1 wave 2 (j>=16 -> S2 partitions 64:128) ----
            for e in range(NP):
                turn_lane(e).dma_start(
                    S2[64:128, e * BQ:(e + 1) * BQ], sAv[:, e, 16:32])

            # ---- stage M (copies on vector; turn-2 wave 1 for e<12 is
            # issued mid-stage: dst = contiguous C2 partitions 0:48) ----
            for e in range(0, NP, 2):
                ps = psum.tile([128, 2 * BQ], F32, name="ps_m", tag="ps")
                nc.tensor.matmul(ps[:, 0:BQ], wm_t[:, e * 128:(e + 1) * 128],
                                 S2[:, e * BQ:(e + 1) * BQ],
                                 start=True, stop=True)
                nc.tensor.matmul(ps[:, BQ:2 * BQ],
                                 wm_t[:, (e + 1) * 128:(e + 2) * 128],
                                 S2[:, (e + 1) * BQ:(e + 2) * BQ],
                                 start=True, stop=True)
                if (e // 2) % 2 == 0:
                    nc.vector.tensor_copy(mid[:, e * BQ:(e + 2) * BQ], ps[:])
                else:
                    nc.scalar.copy(mid[:, e * BQ:(e + 2) * BQ], ps[:])
                if e == 10:
                    for i in range(KO):
                        turn_lane(i).dma_start(
                            C2[0:48, i * BQ:(i + 1) * BQ], midv[:, i, 0:12])

            # ---- turn 2 wave 2 (e>=12 -> C2 partitions 48:96) ----
            for i in range(KO):
                turn_lane(i).dma_start(
                    C2[48:96, i * BQ:(i + 1) * BQ], midv[:, i, 12:24])

            # ---- stage C + output ----
            for i in range(0, KO, 2):
                ps = psum.tile([128, 2 * BQ], F32, name="ps_c", tag="ps")
                nc.tensor.matmul(ps[:, 0:BQ], g_ap,
                                 C2[:, i * BQ:(i + 1) * BQ],
                                 start=True, stop=True)
                nc.tensor.matmul(ps[:, BQ:2 * BQ], g_ap,
                                 C2[:, (i + 1) * BQ:(i + 2) * BQ],
                                 start=True, stop=True)
                if (i // 2) % 2 == 0:
                    nc.vector.tensor_copy(oT[:, i * BQ:(i + 2) * BQ], ps[:])
                else:
                    nc.scalar.copy(oT[:, i * BQ:(i + 2) * BQ], ps[:])
                flush = {6: (0, 0), 14: (8, 1), 22: (16, 0),
                         26: (24, 1), 28: (28, 0), 30: (30, 1)}
                if i in flush:
                    i0, ln = flush[i]
                    (nc.sync if ln == 0 else nc.scalar).dma_start(
                        op[:, i0 * BQ:(i + 2) * BQ],
                        oT[:, i0 * BQ:(i + 2) * BQ])
    nc.finalize()
    return nc


def _get_nc():
    if "nc" not in _CACHE:
        _CACHE["nc"] = _build_nc()
    return _CACHE["nc"]


def _host_weights(W_real, W_imag):
    """F [128,128] (cols 96: zero), G [96,128], Wm [24,128,128] float64."""
    t = np.arange(B).astype(np.float64)
    # F columns (c, e) = (fl,p,e): col = fl*48 + p*24 + e; f = 2e+fl
    F = np.zeros((128, 128))
    for fl in range(2):
        for p in range(2):
            for e in range(NP):
                f = 2 * e + fl
                w = 2 * np.pi * f * t / B
                F[:, fl * 48 + p * 24 + e] = np.cos(w) if p == 0 else -np.sin(w)
    # G rows eh-blocked for contiguous turn-2 wave dsts:
    # row = eh*48 + (q*2+fl)*12 + el, e = eh*12+el, f = 2e+fl;
    # q=0 -> scale*cos, q=1 -> -scale*sin
    G = np.zeros((96, 128))
    scale = np.full(KT, 2.0 / B)
    scale[0] = 1.0 / B
    for eh in range(2):
        for q in range(2):
            for fl in range(2):
                for el in range(12):
                    f = 2 * (eh * 12 + el) + fl
                    w = 2 * np.pi * f * t / B
                    G[eh * 48 + (q * 2 + fl) * 12 + el] = (
                        scale[f] * np.cos(w) if q == 0
                        else -scale[f] * np.sin(w))
    # Wm[e]: rows (fl, p, j) = fl*64+p*32+j; cols (q, fl, i) = q*64+fl*32+i
    Wr = W_real.astype(np.float64)
    Wi = W_imag.astype(np.float64)
    Wm = np.zeros((NP, 128, 128))
    for e in range(NP):
        for fl in range(2):
            f = 2 * e + fl
            r0 = fl * 64
            c0 = fl * 32
            Wrf = Wr[:, :, f].T  # [j, i]
            Wif = Wi[:, :, f].T
            Wm[e, r0:r0 + 32, c0:c0 + 32] = Wrf            # p0 -> q0: Wr
            Wm[e, r0 + 32:r0 + 64, c0:c0 + 32] = Wif       # p1 -> q0: Wi
            Wm[e, r0:r0 + 32, 64 + c0:64 + c0 + 32] = -Wif  # p0 -> q1: -Wi
            Wm[e, r0 + 32:r0 + 64, 64 + c0:64 + c0 + 32] = Wrf  # p1 -> q1
    return F, G, Wm


def kernel(x, W_real, W_imag):
    global LAST_RESULTS
    from concourse.bass_utils import run_bass_kernel_spmd

    x = np.asarray(x, dtype=np.float32)
    F, G, Wm = _host_weights(np.asarray(W_real), np.asarray(W_imag))
    fg_pack = np.zeros((128, 256), np.float16)
    fg_pack[:, 0:128] = F.astype(np.float16)
    fg_pack[0:96, 128:256] = G.astype(np.float16)
    # S2 rows are jh-blocked (row = jh*64 + c*16 + jl, j = jh*16+jl) so
    # turn-1 waves write contiguous partitions; permute Wm rows to match
    rperm = np.empty(128, np.int64)
    for fl in range(2):
        for p in range(2):
            for j in range(KI):
                rperm[(j // 16) * 64 + (fl * 2 + p) * 16 + (j % 16)] = (
                    fl * 64 + p * 32 + j)
    wm_pack = np.ascontiguousarray(
        Wm[:, rperm, :].transpose(1, 0, 2)).reshape(
            128, NP * 128).astype(np.float16)
    x16 = x.astype(np.float16)

    in_maps = []
    for c in range(N_CORES):
        xs = x16[c * BQ:(c + 1) * BQ, :]  # [512, 4096]
        xpk = np.ascontiguousarray(
            xs.reshape(BQ, KI, B).transpose(2, 1, 0)).reshape(128, KI * BQ)
        in_maps.append({"xp": xpk, "fg": fg_pack, "wm": wm_pack})

    nc = _get_nc()
    res = run_bass_kernel_spmd(nc, in_maps, list(range(N_CORES)), trace=TRACE)
    LAST_RESULTS = res

    out = np.empty((BATCH, OUT_F), np.float32)
    for c in range(N_CORES):
        o = np.asarray(res.results[c]["op"])  # [128, KO*BQ] fp16
        out[c * BQ:(c + 1) * BQ, :] = (
            o.reshape(128, KO, BQ).transpose(2, 1, 0)
            .reshape(BQ, OUT_F).astype(np.float32))
    return out
